# revision 1
# baseline (speedup 1.0000x reference)
"""Trainium2 Bass kernel for nn_FactorizedCrossAttention.

Key algebraic facts used (verified against the reference in fp64):
  * The "spatial" and "temporal" branches compute IDENTICAL per-position
    values: cross-attention over text tokens is independent per query row,
    and qt rows equal qs rows (same x row through the same Wq).  Hence
    spatial == temporal exactly.
  * concat([A, A]) @ Wst @ Wo == A @ ((Wst[:D] + Wst[D:]) @ Wo) — so both
    output projections fold into one 1024x1024 matrix Weff.
  * softmax scale (0.125) is folded into K on the host; the padding-mask
    bias is folded into an extra rank-1 accumulation matmul (skipped when
    the mask is all-True, which is the spec'd input).  No max-subtraction:
    scores are O(1) so exp cannot overflow.
  * softmax sums come from the PV matmul via a ones column appended to V
    (even heads) or a separate rank-1 ones matmul (odd heads), so no
    partition-dim reductions are needed.

Sharding: pure data-parallel over (B, T_frames): 32 frames / 8 cores =
4 frames (4096 query rows) per core; K/V/weights replicated.  No
collectives.

Device layout is "transposed activations": X^T, Q^T, A^T all live as
[feature-part, row-free] tiles so every matmul is a natural slice.  Head h
occupies partitions (h%2)*64..+64 of feature chunk h//2; K^T is replicated
on both partition halves so odd heads read lane-aligned operands, and odd
heads' PV output is placed at PSUM base 64 (tile_position) so the
normalized A^T lands on partitions 64..127 without any cross-partition
copies.
"""

import sys

if "/opt/trn_rl_repo" not in sys.path:
    sys.path.insert(0, "/opt/trn_rl_repo")

from contextlib import ExitStack

import ml_dtypes
import numpy as np

import concourse.bass as bass
import concourse.mybir as mybir
import concourse.tile as tile
from concourse import bacc
from concourse.bass_utils import run_bass_kernel_spmd

BF16 = ml_dtypes.bfloat16

D = 1024           # d_model
H = 16             # num heads
G = 4              # query groups
HD = 64            # head dim
HPG = H // G       # heads per group
SCALE = 0.125
B, T, HW, TT = 2, 16, 1024, 77
NCORES = 8
FPC = (B * T) // NCORES      # frames per core = 4
ROWS = FPC * HW              # 4096 query rows per core
RT = 512                     # rows per row-tile
NRT = ROWS // RT             # 8
NK = D // 128                # 8 partition chunks of d_model
VA = HD + 1                  # v columns + ones column

_PROG_CACHE = {}


def _patch_act_tables():
    """Force every activation onto the one table set that contains Exp, Ln
    and Copy together (natural_log_exp_and_others, same 400-interval
    precision).  Without this, bacc's table-load pass alternates between the
    exp-only and ln-only sets, costing a ~1.28us ACT_TABLE_LOAD per switch
    (~330us/core measured)."""
    import concourse.bacc as _bm
    import concourse.hw_specs as _hw
    if getattr(_bm, "_act_tables_patched", False):
        return
    _orig = _hw.get_activation_tables

    def patched(arch):
        t = dict(_orig(arch))
        combo = None
        for name, funcs in t.items():
            if (mybir.ActivationFunctionType.Exp in funcs
                    and mybir.ActivationFunctionType.Ln in funcs):
                combo = name
                break
        if combo is not None:
            for name in list(t):
                if name != combo:
                    t[name] = set()
        return t

    _bm.get_activation_tables = patched
    _bm._act_tables_patched = True

# test.py can flip these for profiling runs
TRACE = False
TRACE_KWARGS = {}
LAST_RESULTS = None


def _build_program(with_bias: bool):
    _patch_act_tables()
    dt = mybir.dt
    nc = bacc.Bacc("TRN2", target_bir_lowering=False, debug=False,
                   num_devices=NCORES)

    xt = nc.dram_tensor("xt", [D, ROWS], dt.bfloat16, kind="ExternalInput").ap()
    wq = nc.dram_tensor("wq", [D, D], dt.bfloat16, kind="ExternalInput").ap()
    weff = nc.dram_tensor("weff", [D, D], dt.bfloat16, kind="ExternalInput").ap()
    # K^T replicated on both partition halves: [128, G*TT]
    kt = nc.dram_tensor("kt", [128, G * TT], dt.bfloat16, kind="ExternalInput").ap()
    # V with a trailing ones column per group: [TT, G*(HD+1)]
    vaug = nc.dram_tensor("vaug", [TT, G * VA], dt.bfloat16, kind="ExternalInput").ap()
    if with_bias:
        biasr = nc.dram_tensor("biasr", [1, TT], dt.bfloat16, kind="ExternalInput").ap()
    out = nc.dram_tensor("out", [ROWS, D], dt.float32, kind="ExternalOutput").ap()

    with tile.TileContext(nc) as tc, ExitStack() as ctx:
        wpool = ctx.enter_context(tc.tile_pool(name="weights", bufs=1))
        xpool = ctx.enter_context(tc.tile_pool(name="xt", bufs=2))
        qpool = ctx.enter_context(tc.tile_pool(name="qt", bufs=2))
        apool = ctx.enter_context(tc.tile_pool(name="at", bufs=2))
        aupool = ctx.enter_context(tc.tile_pool(name="au", bufs=2))
        ppool = ctx.enter_context(tc.tile_pool(name="pt", bufs=3))
        supool = ctx.enter_context(tc.tile_pool(name="sumsb", bufs=6))
        lnpool = ctx.enter_context(tc.tile_pool(name="lnt", bufs=6))
        rpool = ctx.enter_context(tc.tile_pool(name="recip", bufs=6))
        opool = ctx.enter_context(tc.tile_pool(name="osb", bufs=3))
        # 8 PSUM banks: qp(2) + sprb(2, scores+bcast) + ap(2) + op(2)
        qpsum = ctx.enter_context(tc.tile_pool(name="qpsum", bufs=2, space="PSUM"))
        spsum = ctx.enter_context(tc.tile_pool(name="spsum", bufs=2, space="PSUM"))
        apsum = ctx.enter_context(tc.tile_pool(name="apsum", bufs=2, space="PSUM"))
        opsum = ctx.enter_context(tc.tile_pool(name="opsum", bufs=2, space="PSUM"))

        # --- resident weights ---
        wq_t = wpool.tile([128, NK * D], dt.bfloat16, tag="wq")
        weff_t = wpool.tile([128, NK * D], dt.bfloat16, tag="weff")
        for kc in range(NK):
            nc.sync.dma_start(out=wq_t[:, kc * D:(kc + 1) * D],
                              in_=wq[kc * 128:(kc + 1) * 128, :])
            nc.sync.dma_start(out=weff_t[:, kc * D:(kc + 1) * D],
                              in_=weff[kc * 128:(kc + 1) * 128, :])
        kt_t = wpool.tile([128, G * TT], dt.bfloat16, tag="kt")
        nc.sync.dma_start(out=kt_t[:], in_=kt[:, :])
        vaug_t = wpool.tile([TT, G * VA], dt.bfloat16, tag="vaug")
        nc.sync.dma_start(out=vaug_t[:], in_=vaug[:, :])
        ones77_t = wpool.tile([TT, 1], dt.bfloat16, tag="ones77")
        nc.vector.memset(ones77_t[:], 1.0)
        ones64_t = wpool.tile([128, HD], dt.bfloat16, tag="ones64")
        nc.vector.memset(ones64_t[:], 1.0)
        if with_bias:
            bias_t = wpool.tile([1, TT], dt.bfloat16, tag="bias")
            nc.sync.dma_start(out=bias_t[:], in_=biasr[:, :])
            ones_t = wpool.tile([1, RT], dt.bfloat16, tag="ones")
            nc.vector.memset(ones_t[:], 1.0)

        prev = None
        for rt in range(NRT):
            rsl = bass.ts(rt, RT)  # row slice in DRAM

            # --- load X^T row-tile: [1024 din, 512 rows] packed [128, 8*512]
            xt_t = xpool.tile([128, NK * RT], dt.bfloat16, tag="xt")
            for kc in range(NK):
                nc.sync.dma_start(
                    out=xt_t[:, kc * RT:(kc + 1) * RT],
                    in_=xt[kc * 128:(kc + 1) * 128, rsl],
                )

            # --- Q^T = Wq^T @ X^T : [1024 qcols, 512 rows] packed [128, 8*512]
            qt_t = qpool.tile([128, NK * RT], dt.bfloat16, tag="qt")
            for cc in range(NK):
                qp = qpsum.tile([128, RT], dt.float32, tag="qp")
                for kc in range(NK):
                    nc.tensor.matmul(
                        qp[:],
                        lhsT=wq_t[:, kc * D + cc * 128: kc * D + (cc + 1) * 128],
                        rhs=xt_t[:, kc * RT:(kc + 1) * RT],
                        start=(kc == 0), stop=(kc == NK - 1),
                    )
                nc.vector.tensor_copy(qt_t[:, cc * RT:(cc + 1) * RT], qp[:])

            # --- attention per head -> A^T packed [128, 8*512] (bf16)
            # sums ride the PV matmul (ones column for even heads at lane 64,
            # rank-1 ones matmul at lane 32 for odd heads); 1/s = exp(-ln s)
            # on ACT (all three ACT funcs live in one table set, see
            # _patch_act_tables), broadcast along the head's 64 partitions
            # with a rank-1 outer-product matmul, then DVE multiply.
            at_t = apool.tile([128, NK * RT], dt.bfloat16, tag="at")

            def wproj_chunk(prt, pat, rc4):
                # one 128-row output chunk of the PREVIOUS rowtile's Out
                ot = opool.tile([128, D], dt.float32, tag="ot")
                for oc in range(2):
                    op_ = opsum.tile([128, RT], dt.float32, tag="op")
                    for ac in range(NK):
                        nc.tensor.matmul(
                            op_[:],
                            lhsT=pat[:, ac * RT + rc4 * 128: ac * RT + (rc4 + 1) * 128],
                            rhs=weff_t[:, ac * D + oc * 512: ac * D + (oc + 1) * 512],
                            start=(ac == 0), stop=(ac == NK - 1),
                        )
                    nc.vector.tensor_copy(ot[:, oc * 512:(oc + 1) * 512], op_[:])
                nc.sync.dma_start(
                    out=out[prt * RT + rc4 * 128: prt * RT + (rc4 + 1) * 128, :],
                    in_=ot[:],
                )

            for h in range(H):
                g = h // HPG
                cc = h // 2
                po = (h % 2) * HD          # partition base of this head's Q/A
                csl = bass.ds(cc * RT, RT)  # column slice of the packed tiles

                sp = spsum.tile([128, RT], dt.float32, tag="sprb")
                nc.tensor.matmul(
                    sp[0:TT, :],
                    lhsT=kt_t[po:po + HD, g * TT:(g + 1) * TT],
                    rhs=qt_t[po:po + HD, csl],
                    start=True, stop=not with_bias,
                )
                if with_bias:
                    nc.tensor.matmul(
                        sp[0:TT, :], lhsT=bias_t[:, :], rhs=ones_t[:, :],
                        start=False, stop=True,
                    )
                pt = ppool.tile([TT, RT], dt.bfloat16, tag="pt")
                nc.scalar.activation(pt[:], sp[0:TT, :],
                                     mybir.ActivationFunctionType.Exp)

                ap_ = apsum.tile([128, RT], dt.float32, tag="ap")
                if po == 0:
                    nc.tensor.matmul(
                        ap_[0:VA, :],
                        lhsT=vaug_t[:, g * VA:(g + 1) * VA],
                        rhs=pt[:],
                        start=True, stop=True,
                    )
                    slane = HD
                else:
                    nc.tensor.matmul(
                        ap_[HD:2 * HD, :],
                        lhsT=vaug_t[:, g * VA:g * VA + HD],
                        rhs=pt[:],
                        start=True, stop=True,
                    )
                    nc.tensor.matmul(
                        ap_[32:33, :],
                        lhsT=ones77_t[:, :],
                        rhs=pt[:],
                        start=True, stop=True,
                    )
                    slane = 32
                lnt = lnpool.tile([128, RT], dt.float32, tag="lnt")
                nc.scalar.activation(lnt[slane:slane + 1, :],
                                     ap_[slane:slane + 1, :],
                                     mybir.ActivationFunctionType.Ln)
                rc = rpool.tile([128, RT], dt.bfloat16, tag="rc")
                nc.scalar.activation(rc[slane:slane + 1, :],
                                     lnt[slane:slane + 1, :],
                                     mybir.ActivationFunctionType.Exp,
                                     scale=-1.0)
                rb = spsum.tile([128, RT], dt.float32, tag="sprb")
                nc.tensor.matmul(
                    rb[po:po + HD, :],
                    lhsT=ones64_t[slane:slane + 1, :],
                    rhs=rc[slane:slane + 1, :],
                    start=True, stop=True,
                )
                rbs = rpool.tile([128, RT], dt.float32, tag="rbs")
                nc.vector.tensor_copy(rbs[po:po + HD, :], rb[po:po + HD, :])
                nc.vector.tensor_mul(
                    at_t[po:po + HD, csl],
                    ap_[po:po + HD, :], rbs[po:po + HD, :],
                )
                # software pipeline: previous rowtile's Wproj chunks are
                # interleaved between attention heads so the PE has dense
                # independent matmul work during the ACT/DVE softmax chains
                # (keeps the HAM clock gate warm).
                if prev is not None and h % 4 == 3:
                    wproj_chunk(prev[0], prev[1], h // 4)

            prev = (rt, at_t)

        # drain: Wproj of the final rowtile
        for rc4 in range(RT // 128):
            wproj_chunk(prev[0], prev[1], rc4)

    nc.compile()
    return nc


def _get_program(with_bias: bool):
    if with_bias not in _PROG_CACHE:
        _PROG_CACHE[with_bias] = _build_program(with_bias)
    return _PROG_CACHE[with_bias]


def _prep_inputs(x, te, mask, Wq, Wk, Wv, Wo, Wst):
    """Host-side fp32 weight prep + per-core shard maps."""
    K = (te @ Wk).reshape(B, TT, G, HD) * SCALE
    V = (te @ Wv).reshape(B, TT, G, HD)
    Weff = ((Wst[:D] + Wst[D:]) @ Wo).astype(np.float32)

    wq_b = Wq.astype(BF16)
    weff_b = Weff.astype(BF16)
    kt_b, vaug_b, bias_b = [], [], []
    for b in range(B):
        k64 = np.ascontiguousarray(
            K[b].transpose(2, 1, 0)).reshape(HD, G * TT).astype(BF16)
        kt_b.append(np.concatenate([k64, k64], axis=0))
        va = np.concatenate([V[b], np.ones((TT, G, 1), np.float32)], axis=2)
        vaug_b.append(np.ascontiguousarray(va).reshape(TT, G * VA).astype(BF16))
        bias_b.append(np.where(mask[b], 0.0, -30.0).astype(BF16).reshape(1, TT))

    with_bias = not bool(mask.all())
    in_maps = []
    for c in range(NCORES):
        b = c // (NCORES // B)
        fr = (c % (NCORES // B)) * FPC
        xc = x[b, fr:fr + FPC].reshape(ROWS, D).astype(BF16)
        m = {
            "xt": np.ascontiguousarray(xc.T),
            "wq": wq_b,
            "weff": weff_b,
            "kt": kt_b[b],
            "vaug": vaug_b[b],
        }
        if with_bias:
            m["biasr"] = bias_b[b]
        in_maps.append(m)
    return with_bias, in_maps


def kernel(x, text_embeddings, padding_mask, use_mqa=0, use_qk_norm=0,
           Wq=None, Wk=None, Wv=None, Wo=None, Wst=None):
    global LAST_RESULTS
    x = np.asarray(x, np.float32)
    te = np.asarray(text_embeddings, np.float32)
    mask = np.asarray(padding_mask).astype(bool)
    Wq = np.asarray(Wq, np.float32)
    Wk = np.asarray(Wk, np.float32)
    Wv = np.asarray(Wv, np.float32)
    Wo = np.asarray(Wo, np.float32)
    Wst = np.asarray(Wst, np.float32)
    assert x.shape == (B, T, HW, D) and te.shape == (B, TT, D)

    with_bias, in_maps = _prep_inputs(x, te, mask, Wq, Wk, Wv, Wo, Wst)
    nc = _get_program(with_bias)

    res = run_bass_kernel_spmd(nc, in_maps, list(range(NCORES)),
                               trace=TRACE, **TRACE_KWARGS)
    LAST_RESULTS = res

    outp = np.empty((B, T, HW, D), np.float32)
    for c in range(NCORES):
        b = c // (NCORES // B)
        fr = (c % (NCORES // B)) * FPC
        outp[b, fr:fr + FPC] = res.results[c]["out"].reshape(FPC, HW, D)
    return outp



# revision 3
# speedup vs baseline: 1.4221x; 1.4221x over previous
"""Trainium2 Bass kernel for nn_FactorizedCrossAttention.

Key algebraic facts used (verified against the reference in fp64):
  * The "spatial" and "temporal" branches compute IDENTICAL per-position
    values: cross-attention over text tokens is independent per query row,
    and qt rows equal qs rows (same x row through the same Wq).  Hence
    spatial == temporal exactly.
  * concat([A, A]) @ Wst @ Wo == A @ ((Wst[:D] + Wst[D:]) @ Wo) — so both
    output projections fold into one 1024x1024 matrix Weff.
  * softmax scale (0.125) is folded into K on the host; the padding-mask
    bias is folded into an extra rank-1 accumulation matmul (skipped when
    the mask is all-True, which is the spec'd input).  No max-subtraction:
    scores are O(1) so exp cannot overflow.

Softmax bookkeeping (v1 surgery over the 587us baseline):
  * Per-head softmax sums are collected into ONE [16, 512] PSUM tile via
    one-hot selector matmuls (lhsT = E_h [77, 16], col h all-ones),
    accumulated across the 16 heads of a rowtile.  This kills the
    per-head [1,512] ln/exp ACT chain of the baseline (~175us of ACT
    busy + the serial dependency that stalled the PE).
  * One DVE reciprocal [16,512] + one ACT bf16 cast per rowtile.
  * The reciprocal is broadcast to A^T's 128 partitions with ONE selector
    matmul per head-PAIR (lhsT [16, 128]: row 2c -> cols 0..63, row 2c+1
    -> cols 64..127), halving the baseline's rank-1 broadcast count.
  * PV results (A^T unnormalized, one [128, 512] tile per head pair) are
    copied PSUM->SBUF fp32 right after the pair completes so PSUM stays
    within 8 banks; the tail does bcast-matmul + DVE multiply into the
    packed bf16 A^T.

Sharding: pure data-parallel over (B, T_frames): 32 frames / 8 cores =
4 frames (4096 query rows) per core; K/V/weights replicated.  No
collectives.

Device layout is "transposed activations": X^T, Q^T, A^T all live as
[feature-part, row-free] tiles so every matmul is a natural slice.  Head h
occupies partitions (h%2)*64..+64 of feature chunk h//2; K^T is replicated
on both partition halves so odd heads read lane-aligned operands, and odd
heads' PV output is placed at PSUM base 64 (tile_position) so A^T lands on
partitions 64..127 without any cross-partition copies.
"""

import sys

if "/opt/trn_rl_repo" not in sys.path:
    sys.path.insert(0, "/opt/trn_rl_repo")

from contextlib import ExitStack

import ml_dtypes
import numpy as np

import concourse.bass as bass
import concourse.mybir as mybir
import concourse.tile as tile
from concourse import bacc
from concourse.bass_utils import run_bass_kernel_spmd

BF16 = ml_dtypes.bfloat16

D = 1024           # d_model
H = 16             # num heads
G = 4              # query groups
HD = 64            # head dim
HPG = H // G       # heads per group
SCALE = 0.125
B, T, HW, TT = 2, 16, 1024, 77
NCORES = 8
FPC = (B * T) // NCORES      # frames per core = 4
ROWS = FPC * HW              # 4096 query rows per core
RT = 512                     # rows per row-tile
NRT = ROWS // RT             # 8
NK = D // 128                # 8 partition chunks of d_model

_PROG_CACHE = {}


def _patch_act_tables():
    """Force every activation onto the one table set that contains Exp, Ln
    and Copy together (natural_log_exp_and_others, same 400-interval
    precision).  Without this, bacc's table-load pass can alternate between
    table sets, costing a ~1.28us ACT_TABLE_LOAD per switch."""
    import concourse.bacc as _bm
    import concourse.hw_specs as _hw
    if getattr(_bm, "_act_tables_patched", False):
        return
    _orig = _hw.get_activation_tables

    def patched(arch):
        t = dict(_orig(arch))
        combo = None
        for name, funcs in t.items():
            if (mybir.ActivationFunctionType.Exp in funcs
                    and mybir.ActivationFunctionType.Ln in funcs):
                combo = name
                break
        if combo is not None:
            for name in list(t):
                if name != combo:
                    t[name] = set()
        return t

    _bm.get_activation_tables = patched
    _bm._act_tables_patched = True

# test.py can flip these for profiling runs
TRACE = False
TRACE_KWARGS = {}
LAST_RESULTS = None


def _build_program(with_bias: bool):
    _patch_act_tables()
    dt = mybir.dt
    nc = bacc.Bacc("TRN2", target_bir_lowering=False, debug=False,
                   num_devices=NCORES)

    xt = nc.dram_tensor("xt", [D, ROWS], dt.bfloat16, kind="ExternalInput").ap()
    wq = nc.dram_tensor("wq", [D, D], dt.bfloat16, kind="ExternalInput").ap()
    weff = nc.dram_tensor("weff", [D, D], dt.bfloat16, kind="ExternalInput").ap()
    # K^T replicated on both partition halves: [128, G*TT]
    kt = nc.dram_tensor("kt", [128, G * TT], dt.bfloat16, kind="ExternalInput").ap()
    # V: [TT, G*HD]
    v64 = nc.dram_tensor("v64", [TT, G * HD], dt.bfloat16, kind="ExternalInput").ap()
    # per-head one-hot sum selectors: [TT, H*16], block h has col h = 1
    sel = nc.dram_tensor("sel", [TT, H * 16], dt.bfloat16, kind="ExternalInput").ap()
    # per-pair broadcast selectors: [16, 8*128], pair c: row 2c -> cols 0..63,
    # row 2c+1 -> cols 64..127
    selp = nc.dram_tensor("selp", [16, 8 * 128], dt.bfloat16, kind="ExternalInput").ap()
    if with_bias:
        biasr = nc.dram_tensor("biasr", [1, TT], dt.bfloat16, kind="ExternalInput").ap()
    out = nc.dram_tensor("out", [ROWS, D], dt.float32, kind="ExternalOutput").ap()

    with tile.TileContext(nc) as tc, ExitStack() as ctx:
        wpool = ctx.enter_context(tc.tile_pool(name="weights", bufs=1))
        xpool = ctx.enter_context(tc.tile_pool(name="xt", bufs=2))
        qpool = ctx.enter_context(tc.tile_pool(name="qt", bufs=2))
        apool = ctx.enter_context(tc.tile_pool(name="at", bufs=2))
        ppool = ctx.enter_context(tc.tile_pool(name="pt", bufs=4))
        aupool = ctx.enter_context(tc.tile_pool(name="au", bufs=9))
        rpool = ctx.enter_context(tc.tile_pool(name="recip", bufs=2))
        rbpool = ctx.enter_context(tc.tile_pool(name="recipb", bufs=2))
        opool = ctx.enter_context(tc.tile_pool(name="osb", bufs=3))
        # 8 PSUM banks: qp(2) + sprb(2, scores+bcast) + ap(2) + sums(1) + op(1)
        qpsum = ctx.enter_context(tc.tile_pool(name="qpsum", bufs=2, space="PSUM"))
        spsum = ctx.enter_context(tc.tile_pool(name="spsum", bufs=2, space="PSUM"))
        apsum = ctx.enter_context(tc.tile_pool(name="apsum", bufs=2, space="PSUM"))
        supsum = ctx.enter_context(tc.tile_pool(name="supsum", bufs=1, space="PSUM"))
        opsum = ctx.enter_context(tc.tile_pool(name="opsum", bufs=1, space="PSUM"))

        # --- resident weights ---
        wq_t = wpool.tile([128, NK * D], dt.bfloat16, tag="wq")
        weff_t = wpool.tile([128, NK * D], dt.bfloat16, tag="weff")
        for kc in range(NK):
            nc.sync.dma_start(out=wq_t[:, kc * D:(kc + 1) * D],
                              in_=wq[kc * 128:(kc + 1) * 128, :])
            nc.sync.dma_start(out=weff_t[:, kc * D:(kc + 1) * D],
                              in_=weff[kc * 128:(kc + 1) * 128, :])
        kt_t = wpool.tile([128, G * TT], dt.bfloat16, tag="kt")
        nc.sync.dma_start(out=kt_t[:], in_=kt[:, :])
        v64_t = wpool.tile([TT, G * HD], dt.bfloat16, tag="v64")
        nc.sync.dma_start(out=v64_t[:], in_=v64[:, :])
        sel_t = wpool.tile([TT, H * 16], dt.bfloat16, tag="sel")
        nc.sync.dma_start(out=sel_t[:], in_=sel[:, :])
        selp_t = wpool.tile([16, 8 * 128], dt.bfloat16, tag="selp")
        nc.sync.dma_start(out=selp_t[:], in_=selp[:, :])
        if with_bias:
            bias_t = wpool.tile([1, TT], dt.bfloat16, tag="bias")
            nc.sync.dma_start(out=bias_t[:], in_=biasr[:, :])
            ones_t = wpool.tile([1, RT], dt.bfloat16, tag="ones")
            nc.vector.memset(ones_t[:], 1.0)

        prev = None
        for rt in range(NRT):
            rsl = bass.ts(rt, RT)  # row slice in DRAM

            # --- load X^T row-tile: [1024 din, 512 rows] packed [128, 8*512]
            xt_t = xpool.tile([128, NK * RT], dt.bfloat16, tag="xt")
            for kc in range(NK):
                nc.sync.dma_start(
                    out=xt_t[:, kc * RT:(kc + 1) * RT],
                    in_=xt[kc * 128:(kc + 1) * 128, rsl],
                )

            # --- Q^T = Wq^T @ X^T : [1024 qcols, 512 rows] packed [128, 8*512]
            qt_t = qpool.tile([128, NK * RT], dt.bfloat16, tag="qt")
            for cc in range(NK):
                qp = qpsum.tile([128, RT], dt.float32, tag="qp")
                for kc in range(NK):
                    nc.tensor.matmul(
                        qp[:],
                        lhsT=wq_t[:, kc * D + cc * 128: kc * D + (cc + 1) * 128],
                        rhs=xt_t[:, kc * RT:(kc + 1) * RT],
                        start=(kc == 0), stop=(kc == NK - 1),
                    )
                nc.vector.tensor_copy(qt_t[:, cc * RT:(cc + 1) * RT], qp[:])

            # --- attention per head -> A^T packed [128, 8*512] (bf16)
            at_t = apool.tile([128, NK * RT], dt.bfloat16, tag="at")

            def wproj_chunk(prt, pat, rc4):
                # one 128-row output chunk of the PREVIOUS rowtile's Out
                ot = opool.tile([128, D], dt.float32, tag="ot")
                for oc in range(2):
                    op_ = opsum.tile([128, RT], dt.float32, tag="op")
                    for ac in range(NK):
                        nc.tensor.matmul(
                            op_[:],
                            lhsT=pat[:, ac * RT + rc4 * 128: ac * RT + (rc4 + 1) * 128],
                            rhs=weff_t[:, ac * D + oc * 512: ac * D + (oc + 1) * 512],
                            start=(ac == 0), stop=(ac == NK - 1),
                        )
                    nc.vector.tensor_copy(ot[:, oc * 512:(oc + 1) * 512], op_[:])
                nc.sync.dma_start(
                    out=out[prt * RT + rc4 * 128: prt * RT + (rc4 + 1) * 128, :],
                    in_=ot[:],
                )

            sums = supsum.tile([16, RT], dt.float32, tag="sums")
            au_list = []
            for h in range(H):
                g = h // HPG
                cc = h // 2
                po = (h % 2) * HD          # partition base of this head's Q/A
                csl = bass.ds(cc * RT, RT)  # column slice of the packed tiles

                sp = spsum.tile([128, RT], dt.float32, tag="sprb")
                nc.tensor.matmul(
                    sp[0:TT, :],
                    lhsT=kt_t[po:po + HD, g * TT:(g + 1) * TT],
                    rhs=qt_t[po:po + HD, csl],
                    start=True, stop=not with_bias,
                )
                if with_bias:
                    nc.tensor.matmul(
                        sp[0:TT, :], lhsT=bias_t[:, :], rhs=ones_t[:, :],
                        start=False, stop=True,
                    )
                pt = ppool.tile([TT, RT], dt.bfloat16, tag="pt")
                nc.scalar.activation(pt[:], sp[0:TT, :],
                                     mybir.ActivationFunctionType.Exp)

                # head h's softmax sum -> row h of the shared [16, RT] tile
                nc.tensor.matmul(
                    sums[0:16, :],
                    lhsT=sel_t[:, h * 16:(h + 1) * 16],
                    rhs=pt[:],
                    start=(h == 0), stop=(h == H - 1),
                )

                if po == 0:
                    ap_ = apsum.tile([128, RT], dt.float32, tag="ap")
                nc.tensor.matmul(
                    ap_[po:po + HD, :],
                    lhsT=v64_t[:, g * HD:(g + 1) * HD],
                    rhs=pt[:],
                    start=True, stop=True,
                )
                if po != 0:
                    # pair complete: stash unnormalized A^T pair in SBUF fp32
                    au = aupool.tile([128, RT], dt.float32, tag="au")
                    nc.vector.tensor_copy(au[:], ap_[:])
                    au_list.append(au)

                # software pipeline: previous rowtile's Wproj chunks are
                # interleaved between attention heads so the PE has dense
                # independent matmul work during the ACT/DVE softmax chains.
                if prev is not None and h % 4 == 3:
                    wproj_chunk(prev[0], prev[1], h // 4)

            # --- tail: one reciprocal for all 16 heads, then per-pair
            # broadcast + normalize into packed A^T
            rcf = rpool.tile([16, RT], dt.float32, tag="rcf")
            nc.scalar.activation(rcf[0:16, :], sums[0:16, :],
                                 mybir.ActivationFunctionType.Ln)
            rcb = rbpool.tile([16, RT], dt.bfloat16, tag="rcb")
            nc.scalar.activation(rcb[0:16, :], rcf[0:16, :],
                                 mybir.ActivationFunctionType.Exp,
                                 scale=-1.0)
            for c in range(NK):
                rb = spsum.tile([128, RT], dt.float32, tag="sprb")
                nc.tensor.matmul(
                    rb[:],
                    lhsT=selp_t[:, c * 128:(c + 1) * 128],
                    rhs=rcb[0:16, :],
                    start=True, stop=True,
                )
                nc.vector.tensor_mul(
                    at_t[:, c * RT:(c + 1) * RT], au_list[c][:], rb[:],
                )

            prev = (rt, at_t)

        # drain: Wproj of the final rowtile
        for rc4 in range(RT // 128):
            wproj_chunk(prev[0], prev[1], rc4)

    nc.compile()
    return nc


def _get_program(with_bias: bool):
    if with_bias not in _PROG_CACHE:
        _PROG_CACHE[with_bias] = _build_program(with_bias)
    return _PROG_CACHE[with_bias]


def _prep_inputs(x, te, mask, Wq, Wk, Wv, Wo, Wst):
    """Host-side fp32 weight prep + per-core shard maps."""
    K = (te @ Wk).reshape(B, TT, G, HD) * SCALE
    V = (te @ Wv).reshape(B, TT, G, HD)
    Weff = ((Wst[:D] + Wst[D:]) @ Wo).astype(np.float32)

    wq_b = Wq.astype(BF16)
    weff_b = Weff.astype(BF16)

    # per-head one-hot sum selectors [TT, H*16]
    sel_np = np.zeros((TT, H * 16), np.float32)
    for h in range(H):
        sel_np[:, h * 16 + h] = 1.0
    sel_b = sel_np.astype(BF16)
    # per-pair broadcast selectors [16, 8*128]
    selp_np = np.zeros((16, 8 * 128), np.float32)
    for c in range(8):
        selp_np[2 * c, c * 128: c * 128 + 64] = 1.0
        selp_np[2 * c + 1, c * 128 + 64: c * 128 + 128] = 1.0
    selp_b = selp_np.astype(BF16)

    kt_b, v_b, bias_b = [], [], []
    for b in range(B):
        k64 = np.ascontiguousarray(
            K[b].transpose(2, 1, 0)).reshape(HD, G * TT).astype(BF16)
        kt_b.append(np.concatenate([k64, k64], axis=0))
        v_b.append(np.ascontiguousarray(V[b]).reshape(TT, G * HD).astype(BF16))
        bias_b.append(np.where(mask[b], 0.0, -30.0).astype(BF16).reshape(1, TT))

    with_bias = not bool(mask.all())
    in_maps = []
    for c in range(NCORES):
        b = c // (NCORES // B)
        fr = (c % (NCORES // B)) * FPC
        xc = x[b, fr:fr + FPC].reshape(ROWS, D).astype(BF16)
        m = {
            "xt": np.ascontiguousarray(xc.T),
            "wq": wq_b,
            "weff": weff_b,
            "kt": kt_b[b],
            "v64": v_b[b],
            "sel": sel_b,
            "selp": selp_b,
        }
        if with_bias:
            m["biasr"] = bias_b[b]
        in_maps.append(m)
    return with_bias, in_maps


def kernel(x, text_embeddings, padding_mask, use_mqa=0, use_qk_norm=0,
           Wq=None, Wk=None, Wv=None, Wo=None, Wst=None):
    global LAST_RESULTS
    x = np.asarray(x, np.float32)
    te = np.asarray(text_embeddings, np.float32)
    mask = np.asarray(padding_mask).astype(bool)
    Wq = np.asarray(Wq, np.float32)
    Wk = np.asarray(Wk, np.float32)
    Wv = np.asarray(Wv, np.float32)
    Wo = np.asarray(Wo, np.float32)
    Wst = np.asarray(Wst, np.float32)
    assert x.shape == (B, T, HW, D) and te.shape == (B, TT, D)

    with_bias, in_maps = _prep_inputs(x, te, mask, Wq, Wk, Wv, Wo, Wst)
    nc = _get_program(with_bias)

    res = run_bass_kernel_spmd(nc, in_maps, list(range(NCORES)),
                               trace=TRACE, **TRACE_KWARGS)
    LAST_RESULTS = res

    outp = np.empty((B, T, HW, D), np.float32)
    for c in range(NCORES):
        b = c // (NCORES // B)
        fr = (c % (NCORES // B)) * FPC
        outp[b, fr:fr + FPC] = res.results[c]["out"].reshape(FPC, HW, D)
    return outp


# revision 7
# speedup vs baseline: 1.4403x; 1.0127x over previous
"""Trainium2 Bass kernel for nn_FactorizedCrossAttention.

Key algebraic facts used (verified against the reference in fp64):
  * The "spatial" and "temporal" branches compute IDENTICAL per-position
    values: cross-attention over text tokens is independent per query row,
    and qt rows equal qs rows (same x row through the same Wq).  Hence
    spatial == temporal exactly.
  * concat([A, A]) @ Wst @ Wo == A @ ((Wst[:D] + Wst[D:]) @ Wo) — so both
    output projections fold into one 1024x1024 matrix Weff.
  * softmax scale (0.125) is folded into K on the host; the padding-mask
    bias is folded into an extra rank-1 accumulation matmul (skipped when
    the mask is all-True, which is the spec'd input).  No max-subtraction:
    scores are O(1) so exp cannot overflow.

Softmax bookkeeping (v1 surgery over the 587us baseline):
  * Per-head softmax sums are collected into ONE [16, 512] PSUM tile via
    one-hot selector matmuls (lhsT = E_h [77, 16], col h all-ones),
    accumulated across the 16 heads of a rowtile.  This kills the
    per-head [1,512] ln/exp ACT chain of the baseline (~175us of ACT
    busy + the serial dependency that stalled the PE).
  * One DVE reciprocal [16,512] + one ACT bf16 cast per rowtile.
  * The reciprocal is broadcast to A^T's 128 partitions with ONE selector
    matmul per head-PAIR (lhsT [16, 128]: row 2c -> cols 0..63, row 2c+1
    -> cols 64..127), halving the baseline's rank-1 broadcast count.
  * PV results (A^T unnormalized, one [128, 512] tile per head pair) are
    copied PSUM->SBUF fp32 right after the pair completes so PSUM stays
    within 8 banks; the tail does bcast-matmul + DVE multiply into the
    packed bf16 A^T.

Sharding: pure data-parallel over (B, T_frames): 32 frames / 8 cores =
4 frames (4096 query rows) per core; K/V/weights replicated.  No
collectives.

Device layout is "transposed activations": X^T, Q^T, A^T all live as
[feature-part, row-free] tiles so every matmul is a natural slice.  Head h
occupies partitions (h%2)*64..+64 of feature chunk h//2; K^T is replicated
on both partition halves so odd heads read lane-aligned operands, and odd
heads' PV output is placed at PSUM base 64 (tile_position) so A^T lands on
partitions 64..127 without any cross-partition copies.
"""

import sys

if "/opt/trn_rl_repo" not in sys.path:
    sys.path.insert(0, "/opt/trn_rl_repo")

from contextlib import ExitStack

import ml_dtypes
import numpy as np

import concourse.bass as bass
import concourse.mybir as mybir
import concourse.tile as tile
from concourse import bacc
from concourse.bass_utils import run_bass_kernel_spmd

BF16 = ml_dtypes.bfloat16

D = 1024           # d_model
H = 16             # num heads
G = 4              # query groups
HD = 64            # head dim
HPG = H // G       # heads per group
SCALE = 0.125
B, T, HW, TT = 2, 16, 1024, 77
NCORES = 8
FPC = (B * T) // NCORES      # frames per core = 4
ROWS = FPC * HW              # 4096 query rows per core
RT = 512                     # rows per row-tile
NRT = ROWS // RT             # 8
NK = D // 128                # 8 partition chunks of d_model

_PROG_CACHE = {}


def _patch_act_tables():
    """Force every activation onto the one table set that contains Exp, Ln
    and Copy together (natural_log_exp_and_others, same 400-interval
    precision).  Without this, bacc's table-load pass can alternate between
    table sets, costing a ~1.28us ACT_TABLE_LOAD per switch."""
    import concourse.bacc as _bm
    import concourse.hw_specs as _hw
    if getattr(_bm, "_act_tables_patched", False):
        return
    _orig = _hw.get_activation_tables

    def patched(arch):
        t = dict(_orig(arch))
        combo = None
        for name, funcs in t.items():
            if (mybir.ActivationFunctionType.Exp in funcs
                    and mybir.ActivationFunctionType.Ln in funcs):
                combo = name
                break
        if combo is not None:
            for name in list(t):
                if name != combo:
                    t[name] = set()
        return t

    _bm.get_activation_tables = patched
    _bm._act_tables_patched = True

# test.py can flip these for profiling runs
TRACE = False
TRACE_KWARGS = {}
LAST_RESULTS = None


def _build_program(with_bias: bool):
    _patch_act_tables()
    dt = mybir.dt
    nc = bacc.Bacc("TRN2", target_bir_lowering=False, debug=False,
                   num_devices=NCORES)

    xt = nc.dram_tensor("xt", [D, ROWS], dt.bfloat16, kind="ExternalInput").ap()
    wq = nc.dram_tensor("wq", [D, D], dt.bfloat16, kind="ExternalInput").ap()
    weff = nc.dram_tensor("weff", [D, D], dt.bfloat16, kind="ExternalInput").ap()
    # K^T replicated on both partition halves: [128, G*TT]
    kt = nc.dram_tensor("kt", [128, G * TT], dt.bfloat16, kind="ExternalInput").ap()
    # V: [TT, G*HD]
    v64 = nc.dram_tensor("v64", [TT, G * HD], dt.bfloat16, kind="ExternalInput").ap()
    # per-head one-hot sum selectors: [TT, H*16], block h has col h = 1
    sel = nc.dram_tensor("sel", [TT, H * 16], dt.bfloat16, kind="ExternalInput").ap()
    # per-pair broadcast selectors: [16, 8*128], pair c: row 2c -> cols 0..63,
    # row 2c+1 -> cols 64..127
    selp = nc.dram_tensor("selp", [16, 8 * 128], dt.bfloat16, kind="ExternalInput").ap()
    if with_bias:
        biasr = nc.dram_tensor("biasr", [1, TT], dt.bfloat16, kind="ExternalInput").ap()
    out = nc.dram_tensor("out", [ROWS, D], dt.float32, kind="ExternalOutput").ap()

    with tile.TileContext(nc) as tc, ExitStack() as ctx:
        wpool = ctx.enter_context(tc.tile_pool(name="weights", bufs=1))
        xpool = ctx.enter_context(tc.tile_pool(name="xt", bufs=2))
        qpool = ctx.enter_context(tc.tile_pool(name="qt", bufs=2))
        apool = ctx.enter_context(tc.tile_pool(name="at", bufs=2))
        ppool = ctx.enter_context(tc.tile_pool(name="pt", bufs=4))
        aupool = ctx.enter_context(tc.tile_pool(name="au", bufs=9))
        rpool = ctx.enter_context(tc.tile_pool(name="recip", bufs=2))
        rbpool = ctx.enter_context(tc.tile_pool(name="recipb", bufs=2))
        opool = ctx.enter_context(tc.tile_pool(name="osb", bufs=3))
        # 8 PSUM banks: qp(2) + sprb(2, scores+bcast) + ap(2) + sums(1) + op(1)
        qpsum = ctx.enter_context(tc.tile_pool(name="qpsum", bufs=2, space="PSUM"))
        spsum = ctx.enter_context(tc.tile_pool(name="spsum", bufs=2, space="PSUM"))
        apsum = ctx.enter_context(tc.tile_pool(name="apsum", bufs=2, space="PSUM"))
        supsum = ctx.enter_context(tc.tile_pool(name="supsum", bufs=1, space="PSUM"))
        opsum = ctx.enter_context(tc.tile_pool(name="opsum", bufs=1, space="PSUM"))

        # --- resident weights; first rowtile's X^T + Wq lead the DMA queue so
        # the first Qproj chain starts ~8us earlier
        wq_t = wpool.tile([128, NK * D], dt.bfloat16, tag="wq")
        weff_t = wpool.tile([128, NK * D], dt.bfloat16, tag="weff")
        xt0_t = xpool.tile([128, NK * RT], dt.bfloat16, tag="xt")
        for kc in range(NK):
            nc.sync.dma_start(out=xt0_t[:, kc * RT:(kc + 1) * RT],
                              in_=xt[kc * 128:(kc + 1) * 128, 0:RT])
            nc.sync.dma_start(out=wq_t[:, kc * D:(kc + 1) * D],
                              in_=wq[kc * 128:(kc + 1) * 128, :])
        kt_t = wpool.tile([128, G * TT], dt.bfloat16, tag="kt")
        nc.sync.dma_start(out=kt_t[:], in_=kt[:, :])
        v64_t = wpool.tile([TT, G * HD], dt.bfloat16, tag="v64")
        nc.sync.dma_start(out=v64_t[:], in_=v64[:, :])
        sel_t = wpool.tile([TT, H * 16], dt.bfloat16, tag="sel")
        nc.sync.dma_start(out=sel_t[:], in_=sel[:, :])
        selp_t = wpool.tile([16, 8 * 128], dt.bfloat16, tag="selp")
        nc.sync.dma_start(out=selp_t[:], in_=selp[:, :])
        if with_bias:
            bias_t = wpool.tile([1, TT], dt.bfloat16, tag="bias")
            nc.sync.dma_start(out=bias_t[:], in_=biasr[:, :])
            ones_t = wpool.tile([1, RT], dt.bfloat16, tag="ones")
            nc.vector.memset(ones_t[:], 1.0)
        for kc in range(NK):
            nc.sync.dma_start(out=weff_t[:, kc * D:(kc + 1) * D],
                              in_=weff[kc * 128:(kc + 1) * 128, :])

        prev = None
        for rt in range(NRT):
            rsl = bass.ts(rt, RT)  # row slice in DRAM

            # --- load X^T row-tile: [1024 din, 512 rows] packed [128, 8*512]
            if rt == 0:
                xt_t = xt0_t
            else:
                xt_t = xpool.tile([128, NK * RT], dt.bfloat16, tag="xt")
                for kc in range(NK):
                    nc.sync.dma_start(
                        out=xt_t[:, kc * RT:(kc + 1) * RT],
                        in_=xt[kc * 128:(kc + 1) * 128, rsl],
                    )

            # --- Q^T = Wq^T @ X^T : [1024 qcols, 512 rows] packed [128, 8*512]
            qt_t = qpool.tile([128, NK * RT], dt.bfloat16, tag="qt")
            for cc in range(NK):
                qp = qpsum.tile([128, RT], dt.float32, tag="qp")
                for kc in range(NK):
                    nc.tensor.matmul(
                        qp[:],
                        lhsT=wq_t[:, kc * D + cc * 128: kc * D + (cc + 1) * 128],
                        rhs=xt_t[:, kc * RT:(kc + 1) * RT],
                        start=(kc == 0), stop=(kc == NK - 1),
                    )
                nc.vector.tensor_copy(qt_t[:, cc * RT:(cc + 1) * RT], qp[:])

            # --- attention per head -> A^T packed [128, 8*512] (bf16)
            at_t = apool.tile([128, NK * RT], dt.bfloat16, tag="at")

            def wproj_gen(prt, pat):
                # previous rowtile's Out projection as a stream of small
                # PE batches, pulled between attention heads so the PE has
                # dense independent work during ACT/DVE latencies.  PSUM
                # double-buffers by alternating the op pool with the (idle
                # during the head loop) Qproj pool.
                for rc4 in range(4):
                    ot = opool.tile([128, D], dt.float32, tag="ot")
                    for oc in range(2):
                        pool = opsum if (rc4 * 2 + oc) % 2 == 0 else qpsum
                        op_ = pool.tile([128, RT], dt.float32,
                                        tag="op" if pool is opsum else "qp")
                        for ac in range(4):
                            nc.tensor.matmul(
                                op_[:],
                                lhsT=pat[:, ac * RT + rc4 * 128: ac * RT + (rc4 + 1) * 128],
                                rhs=weff_t[:, ac * D + oc * 512: ac * D + (oc + 1) * 512],
                                start=(ac == 0), stop=False,
                            )
                        yield
                        for ac in range(4, NK):
                            nc.tensor.matmul(
                                op_[:],
                                lhsT=pat[:, ac * RT + rc4 * 128: ac * RT + (rc4 + 1) * 128],
                                rhs=weff_t[:, ac * D + oc * 512: ac * D + (oc + 1) * 512],
                                start=False, stop=(ac == NK - 1),
                            )
                        nc.vector.tensor_copy(ot[:, oc * 512:(oc + 1) * 512], op_[:])
                        yield
                    nc.sync.dma_start(
                        out=out[prt * RT + rc4 * 128: prt * RT + (rc4 + 1) * 128, :],
                        in_=ot[:],
                    )

            wops = wproj_gen(prev[0], prev[1]) if prev is not None else None

            _done = object()

            def drain(k):
                if wops is None:
                    return
                for _ in range(k):
                    if next(wops, _done) is _done:
                        break

            sums = supsum.tile([16, RT], dt.float32, tag="sums")
            au_list = []
            pair_ap = [None]

            def head_front(h):
                g = h // HPG
                po = (h % 2) * HD
                csl = bass.ds((h // 2) * RT, RT)
                sp = spsum.tile([128, RT], dt.float32, tag="sprb")
                nc.tensor.matmul(
                    sp[0:TT, :],
                    lhsT=kt_t[po:po + HD, g * TT:(g + 1) * TT],
                    rhs=qt_t[po:po + HD, csl],
                    start=True, stop=not with_bias,
                )
                if with_bias:
                    nc.tensor.matmul(
                        sp[0:TT, :], lhsT=bias_t[:, :], rhs=ones_t[:, :],
                        start=False, stop=True,
                    )
                pt = ppool.tile([TT, RT], dt.bfloat16, tag="pt")
                nc.scalar.activation(pt[:], sp[0:TT, :],
                                     mybir.ActivationFunctionType.Exp)
                return pt

            def head_back(h, pt):
                g = h // HPG
                po = (h % 2) * HD
                # head h's softmax sum -> row h of the shared [16, RT] tile
                nc.tensor.matmul(
                    sums[0:16, :],
                    lhsT=sel_t[:, h * 16:(h + 1) * 16],
                    rhs=pt[:],
                    start=(h == 0), stop=(h == H - 1),
                )
                if po == 0:
                    ap_ = apsum.tile([128, RT], dt.float32, tag="ap")
                    pair_ap[0] = ap_
                nc.tensor.matmul(
                    pair_ap[0][po:po + HD, :],
                    lhsT=v64_t[:, g * HD:(g + 1) * HD],
                    rhs=pt[:],
                    start=True, stop=True,
                )
                if po != 0:
                    # pair complete: stash unnormalized A^T pair in SBUF fp32
                    au = aupool.tile([128, RT], dt.float32, tag="au")
                    nc.vector.tensor_copy(au[:], pair_ap[0][:])
                    au_list.append(au)

            # software-pipelined head loop: exp(h) overlaps qk(h+1) and the
            # previous head's sums/PV plus a Wproj batch
            prev_pt = None
            for h in range(H):
                pt = head_front(h)
                if prev_pt is not None:
                    head_back(h - 1, prev_pt)
                prev_pt = pt
                if h >= 2:
                    drain(1)
            head_back(H - 1, prev_pt)

            # --- tail: one reciprocal for all 16 heads, then per-pair
            # broadcast + normalize into packed A^T
            rcf = rpool.tile([16, RT], dt.float32, tag="rcf")
            nc.scalar.activation(rcf[0:16, :], sums[0:16, :],
                                 mybir.ActivationFunctionType.Ln)
            rcb = rbpool.tile([16, RT], dt.bfloat16, tag="rcb")
            nc.scalar.activation(rcb[0:16, :], rcf[0:16, :],
                                 mybir.ActivationFunctionType.Exp,
                                 scale=-1.0)
            drain(2)
            for c in range(NK):
                rb = spsum.tile([128, RT], dt.float32, tag="sprb")
                nc.tensor.matmul(
                    rb[:],
                    lhsT=selp_t[:, c * 128:(c + 1) * 128],
                    rhs=rcb[0:16, :],
                    start=True, stop=True,
                )
                nc.vector.tensor_mul(
                    at_t[:, c * RT:(c + 1) * RT], au_list[c][:], rb[:],
                )
            drain(99)

            prev = (rt, at_t)

        # drain: Wproj of the final rowtile
        for _ in wproj_gen(prev[0], prev[1]):
            pass

    nc.compile()
    return nc


def _get_program(with_bias: bool):
    if with_bias not in _PROG_CACHE:
        _PROG_CACHE[with_bias] = _build_program(with_bias)
    return _PROG_CACHE[with_bias]


def _prep_inputs(x, te, mask, Wq, Wk, Wv, Wo, Wst):
    """Host-side fp32 weight prep + per-core shard maps."""
    K = (te @ Wk).reshape(B, TT, G, HD) * SCALE
    V = (te @ Wv).reshape(B, TT, G, HD)
    Weff = ((Wst[:D] + Wst[D:]) @ Wo).astype(np.float32)

    wq_b = Wq.astype(BF16)
    weff_b = Weff.astype(BF16)

    # per-head one-hot sum selectors [TT, H*16]
    sel_np = np.zeros((TT, H * 16), np.float32)
    for h in range(H):
        sel_np[:, h * 16 + h] = 1.0
    sel_b = sel_np.astype(BF16)
    # per-pair broadcast selectors [16, 8*128]
    selp_np = np.zeros((16, 8 * 128), np.float32)
    for c in range(8):
        selp_np[2 * c, c * 128: c * 128 + 64] = 1.0
        selp_np[2 * c + 1, c * 128 + 64: c * 128 + 128] = 1.0
    selp_b = selp_np.astype(BF16)

    kt_b, v_b, bias_b = [], [], []
    for b in range(B):
        k64 = np.ascontiguousarray(
            K[b].transpose(2, 1, 0)).reshape(HD, G * TT).astype(BF16)
        kt_b.append(np.concatenate([k64, k64], axis=0))
        v_b.append(np.ascontiguousarray(V[b]).reshape(TT, G * HD).astype(BF16))
        bias_b.append(np.where(mask[b], 0.0, -30.0).astype(BF16).reshape(1, TT))

    with_bias = not bool(mask.all())
    in_maps = []
    for c in range(NCORES):
        b = c // (NCORES // B)
        fr = (c % (NCORES // B)) * FPC
        xc = x[b, fr:fr + FPC].reshape(ROWS, D).astype(BF16)
        m = {
            "xt": np.ascontiguousarray(xc.T),
            "wq": wq_b,
            "weff": weff_b,
            "kt": kt_b[b],
            "v64": v_b[b],
            "sel": sel_b,
            "selp": selp_b,
        }
        if with_bias:
            m["biasr"] = bias_b[b]
        in_maps.append(m)
    return with_bias, in_maps


def kernel(x, text_embeddings, padding_mask, use_mqa=0, use_qk_norm=0,
           Wq=None, Wk=None, Wv=None, Wo=None, Wst=None):
    global LAST_RESULTS
    x = np.asarray(x, np.float32)
    te = np.asarray(text_embeddings, np.float32)
    mask = np.asarray(padding_mask).astype(bool)
    Wq = np.asarray(Wq, np.float32)
    Wk = np.asarray(Wk, np.float32)
    Wv = np.asarray(Wv, np.float32)
    Wo = np.asarray(Wo, np.float32)
    Wst = np.asarray(Wst, np.float32)
    assert x.shape == (B, T, HW, D) and te.shape == (B, TT, D)

    with_bias, in_maps = _prep_inputs(x, te, mask, Wq, Wk, Wv, Wo, Wst)
    nc = _get_program(with_bias)

    res = run_bass_kernel_spmd(nc, in_maps, list(range(NCORES)),
                               trace=TRACE, **TRACE_KWARGS)
    LAST_RESULTS = res

    outp = np.empty((B, T, HW, D), np.float32)
    for c in range(NCORES):
        b = c // (NCORES // B)
        fr = (c % (NCORES // B)) * FPC
        outp[b, fr:fr + FPC] = res.results[c]["out"].reshape(FPC, HW, D)
    return outp
